# revision 29
# baseline (speedup 1.0000x reference)
"""Causal multi-head attention on 8 Trainium2 NeuronCores.

Problem (hardcoded): B=4, S=2048, D=1024, H=16, DH=64, fp32.
  q/k/v = x @ W.T + b ; heads split; scores = q k^T / sqrt(DH), causal
  mask, softmax, out = attn @ v, merge heads.

Sharding: data-parallel over batch (4) x tensor-parallel over head
groups (2).  Core c handles batch b = c % 4 and heads
[8*(c//4), 8*(c//4)+8).  Each core gets x[b] and the 512-row slice of
Wq/Wk/Wv (+bias) for its head group, returns out[b, :, 512*hg:+512].
No collectives needed; host scatters inputs / gathers outputs.

Per-core kernel design (Tile framework):
  - x^T and W^T built on-chip with PE transposes (fp32 has no DMA
    transpose path).
  - Projections computed in transposed form: Q^T,K^T = W^T.T @ x^T
    ([dout, s] layout) and V = x^T.T @ W^T ([s, dout] layout), with the
    bias folded in as an extra rank-1 matmul (ones vector x bias).
    Matmul inputs are bitcast to float32r: full-rate (1 cyc/row) on the
    PE at N>=512 vs 4 cyc/row for plain fp32.
  - Scores computed transposed, S^T[k, q] = K_h Q_h^T, one 128-row key
    tile x 1024-wide query chunk at a time, accumulated in PSUM.
  - Softmax without a max pass: scores ~ N(0,1) (max ~6 sigma over the
    whole tensor), so exp((q.k)*0.125) cannot overflow; softmax is
    shift-invariant so the result is exact.  The 1/sqrt(DH) scale rides
    the ACT activation's free scale.  Causal masking only affects the
    diagonal 128x128 block of each (key-tile, query-chunk) pair: a
    precomputed additive mask (-1e30 above... below diag) is added in
    PSUM.  exp() output is written as bf16 (attention weights in [0,
    e^6]) straight to SBUF.
  - attn @ V: attn^T tiles are the stationary operand (bf16 -> fast
    weight load), V tiles [128, 65] the moving operand, where column 64
    is ones so the PE accumulates the softmax denominator alongside.
    Output [q, 64+1] accumulates over key tiles in PSUM.
  - Finalize: reciprocal of column 64, per-partition scalar multiply,
    DMA out.
"""

import numpy as np

# Full problem shapes.
B, S, D, H, DH = 4, 2048, 1024, 16, 64
TP = 2
DP = 4
D_LOC = D // TP  # 512
H_LOC = H // TP  # 8

NEG = -1.0e30

# dtype for the fp32-ish matmuls: "float32r" (TF32-like, full PE rate at
# N>=256) or "float32" (exact, 4 cycles/row).
MM_DTYPE = "float32r"

# Knobs (test.py may override before first kernel() call).
RUN_OPTS: dict = {}
LAST_RESULT = None

_NC_CACHE: dict = {}



def _legalize_waits(nc, mybir):
    """Spill excess sync waits onto NoOps inserted before the instruction.

    Walrus enforces per-instruction sync-wait capacities (Matmult fuses
    LDWEIGHTS and has a single slot; most others have two).  Tile's wait
    assignment can exceed that when an instruction joins several
    semaphore domains.  Moving waits to a same-engine NoOp immediately
    before the instruction is semantics-preserving: the engine's
    sequencer executes them in order.
    """
    caps = {}
    ctr = [0]
    for fn in nc.m.functions:
        for blk in fn.blocks:
            insts = list(blk.instructions)
            out = []
            changed = False
            for inst in insts:
                si = inst.sync_info
                waits = list(si.on_wait) if si is not None and si.on_wait else []
                cap = caps.get(str(inst.opcode), 1)
                if len(waits) > cap:
                    excess = waits[: len(waits) - cap]
                    keep = waits[len(waits) - cap :]
                    for w in excess:
                        ev = mybir.InstEventSemaphore(
                            name=f"waitnop_{ctr[0]}",
                            opcode="EventSemaphore",
                            engine=inst.engine,
                            ins=[],
                            outs=[],
                            sync_info=mybir.SyncInfo(on_wait=[w], on_update=[]),
                        )
                        ctr[0] += 1
                        out.append(ev)
                    si.on_wait = keep
                    inst.sync_info = si
                    changed = True
                out.append(inst)
            if changed:
                blk.instructions = out
    return ctr[0]


def _build_nc(s=S, d_in=D, d_loc=D_LOC, h_loc=H_LOC, dh=DH, legalize=True):
    """Build the per-core Bass program. All 8 cores run this SPMD."""
    from contextlib import ExitStack

    import concourse.bass as bass
    import concourse.mybir as mybir
    import concourse.tile as tile

    f32 = mybir.dt.float32
    f32r = getattr(mybir.dt, MM_DTYPE)
    bf16 = mybir.dt.bfloat16
    EXP = mybir.ActivationFunctionType.Exp

    assert s % 512 == 0 and d_in % 128 == 0 and d_loc % 128 == 0
    assert dh == 64 and d_loc == h_loc * dh
    KD = d_in // 128       # contraction k-tiles for projections
    NM = d_loc // 128      # dout m-tiles (4)
    NSB = s // 512         # s superblocks for projections
    NKT = s // 128         # key tiles (16)
    QC = min(1024, s)      # query chunk width
    NJC = s // QC          # query chunks (2)
    NJJ = QC // 128        # q-tiles per chunk (8)
    SCALE = 1.0 / float(np.sqrt(dh))

    nc = bass.Bass()

    x_d = nc.dram_tensor("x", [s, d_in], f32, kind="ExternalInput")
    wq_d = nc.dram_tensor("wq", [d_loc, d_in], f32, kind="ExternalInput")
    wk_d = nc.dram_tensor("wk", [d_loc, d_in], f32, kind="ExternalInput")
    wv_d = nc.dram_tensor("wv", [d_loc, d_in], f32, kind="ExternalInput")
    bq_d = nc.dram_tensor("bq", [d_loc], f32, kind="ExternalInput")
    bk_d = nc.dram_tensor("bk", [d_loc], f32, kind="ExternalInput")
    bv_d = nc.dram_tensor("bv", [d_loc], f32, kind="ExternalInput")
    out_d = nc.dram_tensor("out", [s, d_loc], f32, kind="ExternalOutput")

    import ml_dtypes

    id_np = np.eye(128, dtype=np.float32)
    # Multiplicative causal mask for the diagonal block of attn^T[k, q]:
    # valid (keep) where k <= q i.e. row <= col.
    mask_np = np.where(
        np.arange(128)[:, None] <= np.arange(128)[None, :], 1.0, 0.0
    ).astype(ml_dtypes.bfloat16)
    id_dram = nc.inline_tensor(id_np, name="id128")
    mask_dram = nc.inline_tensor(mask_np, name="cmask01")

    with tile.TileContext(nc) as tc, ExitStack() as ctx:
        persist = ctx.enter_context(tc.tile_pool(name="persist", bufs=1))
        proj_sb = tc.alloc_tile_pool(name="proj_sb", bufs=1)
        proj_ps = tc.alloc_tile_pool(name="proj_ps", bufs=1, space="PSUM")

        # ---- constants ----
        ident = persist.tile([128, 128], f32)
        nc.sync.dma_start(out=ident, in_=id_dram[:])
        cmask = persist.tile([128, 128], bf16)
        nc.sync.dma_start(out=cmask, in_=mask_dram[:])
        dve_scr = persist.tile([1, 8], f32)
        ones_st = persist.tile([1, 512], f32)
        nc.vector.memset(ones_st, 1.0)
        ones_r = persist.tile([1, 512], f32r)
        nc.vector.tensor_copy(out=ones_r, in_=ones_st)
        bias_st = persist.tile([1, 3, d_loc], f32)
        bias_sb = persist.tile([1, 3, d_loc], f32r)
        for i, b_d in enumerate((bq_d, bk_d, bv_d)):
            nc.sync.dma_start(out=bias_st[:, i, :], in_=b_d[:].unsqueeze(0))
            nc.vector.tensor_copy(out=bias_sb[:, i, :], in_=bias_st[:, i, :])

        # ---- persistent activations ----
        qt_sb = persist.tile([128, NM, s], f32r)      # Q^T  [dout, s]
        kt_sb = persist.tile([128, NM, s], f32r)      # K^T  [dout, s]
        v_sb = persist.tile([128, NKT, h_loc, dh + 1], bf16)  # V (+ones col)
        nc.vector.memset(v_sb, 1.0)

        # Matmult lowers to an S3_LW instruction with a SINGLE sync-wait
        # slot, and DVE/ACT/DMA instructions have two.  All the helpers
        # below exist to keep every instruction within its wait budget:
        # tiny PE "touch" matmuls absorb DMA-completion waits so real
        # matmuls only ever join on one semaphore.
        dummy_ps = proj_ps.tile([128, 8], f32, name="dummy_ps", tag="dummy")

        def pe_touch(ap):
            nc.tensor.matmul(
                dummy_ps[0:1, 0:1],
                lhsT=ap[0:1, 0:1],
                rhs=ap[0:1, 0:1],
                start=True,
                stop=True,
                skip_group_check=True,
            )

        # Warm-up: consume ident (PE) and cmask (DVE) alone so their DMA
        # waits are absorbed before any dual-dependency instruction.
        tp0 = proj_ps.tile([128, 128], f32, name="tp0", tag="tp", bufs=4)
        nc.tensor.matmul(tp0, lhsT=ident, rhs=ident, start=True, stop=True)
        nc.vector.tensor_copy(out=dve_scr[0:1, 0:1], in_=cmask[0:1, 0:1])

        # ---- W^T via PE identity-matmul transpose ----
        wt_tiles = []
        for wi, w_d in enumerate((wq_d, wk_d, wv_d)):
            wnat = proj_sb.tile(
                [128, NM, d_in], f32, name=f"wnat{wi}", tag="wnat", bufs=2
            )
            for m in range(NM):
                nc.sync.dma_start(
                    out=wnat[:, m, :], in_=w_d[128 * m : 128 * (m + 1), :]
                )
                pe_touch(wnat[:, m, :])
            wt = proj_sb.tile(
                [128, KD, d_loc], f32r, name=f"wt{wi}", tag="wt", bufs=3
            )
            for kd in range(KD):
                for m in range(NM):
                    tp = proj_ps.tile(
                        [128, 128], f32, name=f"tpw{wi}_{kd}_{m}", tag="tp", bufs=4
                    )
                    nc.tensor.matmul(
                        tp,
                        lhsT=wnat[:, m, 128 * kd : 128 * (kd + 1)],
                        rhs=ident,
                        start=True,
                        stop=True,
                    )
                    nc.vector.tensor_copy(
                        out=wt[:, kd, 128 * m : 128 * (m + 1)], in_=tp
                    )
            wt_tiles.append(wt)
        wqt, wkt, wvt = wt_tiles

        # ---- projections, one 512-row superblock of s at a time ----
        for sb in range(NSB):
            xt = proj_sb.tile([128, KD, 512], f32r, name=f"xt{sb}", tag="xt", bufs=1)
            for t in range(4):
                xnat = proj_sb.tile(
                    [128, d_in], f32, name=f"xnat{sb}_{t}", tag="xnat", bufs=2
                )
                nc.sync.dma_start(
                    out=xnat, in_=x_d[sb * 512 + 128 * t : sb * 512 + 128 * (t + 1), :]
                )
                pe_touch(xnat)
                for kd in range(KD):
                    tp = proj_ps.tile(
                        [128, 128], f32, name=f"tpx{sb}_{t}_{kd}", tag="tp", bufs=4
                    )
                    nc.tensor.matmul(
                        tp,
                        lhsT=xnat[:, 128 * kd : 128 * (kd + 1)],
                        rhs=ident,
                        start=True,
                        stop=True,
                    )
                    nc.vector.tensor_copy(
                        out=xt[:, kd, 128 * t : 128 * (t + 1)], in_=tp
                    )

            # Q^T, K^T m-tiles: [dout 128, s 512] = sum_kd W^T.T @ x^T
            for wt, dest, bi in ((wqt, qt_sb, 0), (wkt, kt_sb, 1)):
                for m in range(NM):
                    ps = proj_ps.tile(
                        [128, 512], f32, name=f"psp{sb}_{bi}_{m}", tag="mm512", bufs=2
                    )
                    for kd in range(KD):
                        nc.tensor.matmul(
                            ps,
                            lhsT=wt[:, kd, 128 * m : 128 * (m + 1)],
                            rhs=xt[:, kd, :],
                            start=(kd == 0),
                            stop=False,
                        )
                    nc.tensor.matmul(
                        ps,
                        lhsT=bias_sb[:, bi, 128 * m : 128 * (m + 1)],
                        rhs=ones_r[:, :],
                        start=False,
                        stop=True,
                    )
                    nc.vector.tensor_copy(
                        out=dest[:, m, 512 * sb : 512 * (sb + 1)], in_=ps
                    )

            # V s-tiles: [s 128, dout 512] = sum_kd x^T.T @ W^T
            for t in range(4):
                kt_idx = 4 * sb + t
                ps = proj_ps.tile(
                    [128, d_loc], f32, name=f"psv{sb}_{t}", tag="mm512", bufs=2
                )
                for kd in range(KD):
                    nc.tensor.matmul(
                        ps,
                        lhsT=xt[:, kd, 128 * t : 128 * (t + 1)],
                        rhs=wvt[:, kd, :],
                        start=(kd == 0),
                        stop=False,
                    )
                nc.tensor.matmul(
                    ps,
                    lhsT=ones_r[:, 0:128],
                    rhs=bias_sb[:, 2, :],
                    start=False,
                    stop=True,
                )
                # strided copy into per-head [dh] slots (col dh stays ones)
                nc.vector.tensor_copy(
                    out=v_sb[:, kt_idx, :, 0:dh],
                    in_=ps.rearrange("p (h c) -> p h c", c=dh),
                )

        proj_sb.release()
        proj_ps.release()

        attn_sb = ctx.enter_context(tc.tile_pool(name="attn_sb", bufs=1))
        sc_ps_pool = ctx.enter_context(
            tc.tile_pool(name="sc_ps", bufs=1, space="PSUM")
        )
        oa_ps_pool = ctx.enter_context(
            tc.tile_pool(name="oa_ps", bufs=1, space="PSUM")
        )

        # ---- attention ----
        # Wait-budget bookkeeping (see comment at pe_touch): the scores
        # PSUM tile is read ONLY by the exp activation; the output
        # accumulator PSUM tile is read ONLY by one ACT copy; diagonal
        # masking happens post-exp on a separate bf16 tile so attn@V
        # matmuls join on a single semaphore (ACT for the plain tiles,
        # DVE for the masked diagonal tile).
        for jc in range(NJC):  # query chunks of QC
            for h in range(h_loc):
                pbase = 64 * (h % 2)
                mblk = h // 2
                i_max = NJJ * jc + (NJJ - 1)  # last key tile with any valid q
                oa = oa_ps_pool.tile(
                    [128, 1024], f32, name=f"oa{jc}_{h}", tag="oa", bufs=2
                )
                # per-PSUM-bank first/last matmul bookkeeping for start/stop.
                # Order i=0's matmuls non-diagonal-first so the first matmul
                # into each bank depends only on the ACT semaphore.
                def jj_order(i):
                    jj0 = max(0, i - NJJ * jc)
                    jd = i - NJJ * jc  # diagonal jj (may be out of range)
                    jjs = [j for j in range(jj0, NJJ) if j != jd]
                    if jj0 <= jd < NJJ:
                        pos = 1 if len(jjs) >= 1 else 0
                        jjs.insert(pos, jd)
                    return jjs

                mm_sched: dict = {}
                for i in range(i_max + 1):
                    for jj in jj_order(i):
                        mm_sched.setdefault(jj // 4, []).append((i, jj))
                first_mm = {b: v[0] for b, v in mm_sched.items()}
                last_mm = {b: v[-1] for b, v in mm_sched.items()}

                for i in range(i_max + 1):
                    jj0 = max(0, i - NJJ * jc)
                    jd = i - NJJ * jc
                    qv0 = 128 * jj0
                    sc = sc_ps_pool.tile(
                        [128, QC], f32, name=f"sc{jc}_{h}_{i}", tag="sc", bufs=2
                    )
                    kt_lhs = kt_sb[
                        pbase : pbase + dh,
                        mblk,
                        128 * i : 128 * (i + 1),
                    ]
                    for half in range(QC // 512):
                        if 512 * (half + 1) <= qv0:
                            continue  # fully masked half
                        nc.tensor.matmul(
                            sc[:, 512 * half : 512 * (half + 1)],
                            lhsT=kt_lhs,
                            rhs=qt_sb[
                                pbase : pbase + dh,
                                mblk,
                                QC * jc + 512 * half : QC * jc + 512 * (half + 1),
                            ],
                            start=True,
                            stop=True,
                        )
                    at = attn_sb.tile(
                        [128, QC], bf16, name=f"at{jc}_{h}_{i}", tag="at", bufs=3
                    )
                    nc.scalar.activation(
                        out=at[:, qv0:QC], in_=sc[:, qv0:QC], func=EXP, scale=SCALE
                    )
                    # causal mask on the diagonal block (post-exp, bf16)
                    if jj0 <= jd < NJJ:
                        at_m = attn_sb.tile(
                            [128, 128], bf16, name=f"atm{jc}_{h}_{i}",
                            tag="atm", bufs=2,
                        )
                        nc.vector.tensor_mul(
                            out=at_m,
                            in0=at[:, 128 * jd : 128 * (jd + 1)],
                            in1=cmask,
                        )
                    vt = v_sb[:, i, h, :]  # [128, dh+1] bf16
                    for jj in jj_order(i):
                        bank = jj // 4
                        col = 512 * bank + 65 * (jj % 4)
                        lhs = at_m if jj == jd else at[:, 128 * jj : 128 * (jj + 1)]
                        nc.tensor.matmul(
                            oa[:, col : col + 65],
                            lhsT=lhs,
                            rhs=vt,
                            start=(first_mm[bank] == (i, jj)),
                            stop=(last_mm[bank] == (i, jj)),
                        )

                # finalize: one ACT copy drains the accumulator (sole oa
                # reader), then DVE divides by the denominator column.
                ot_raw = attn_sb.tile(
                    [128, 2, 4, dh + 1], f32, name=f"otr{jc}_{h}", tag="otr", bufs=2
                )
                nc.scalar.copy(
                    out=ot_raw,
                    in_=oa.rearrange("p (b g) -> p b g", b=2)[:, :, 0 : 4 * 65]
                    .rearrange("p b (j c) -> p b j c", c=65),
                )
                ot = attn_sb.tile(
                    [128, NJJ, dh], f32, name=f"ot{jc}_{h}", tag="ot", bufs=3
                )
                for jj in range(NJJ):
                    rec = attn_sb.tile(
                        [128, 1], f32, name=f"rec{jc}_{h}_{jj}", tag="rec", bufs=4
                    )
                    nc.vector.reciprocal(
                        rec, ot_raw[:, jj // 4, jj % 4, dh : dh + 1]
                    )
                    nc.vector.tensor_scalar_mul(
                        out=ot[:, jj, :],
                        in0=ot_raw[:, jj // 4, jj % 4, 0:dh],
                        scalar1=rec,
                    )
                nc.sync.dma_start(
                    out=out_d[QC * jc : QC * (jc + 1), dh * h : dh * (h + 1)]
                    .rearrange("(jj p) c -> p jj c", p=128),
                    in_=ot,
                )

    if legalize:
        _legalize_waits(nc, mybir)
    nc.finalize()
    return nc


class _Runner:
    """Caches the compiled SPMD executable across kernel() calls.

    Mirrors concourse.bass2jax.run_bass_via_pjrt's multi-core path, but
    keeps the jitted callable (and thus the NEFF executable) alive so
    repeated calls don't re-trace/re-compile.  Supports running the NEFF
    n_iters times back-to-back inside one jit call (the bass_exec
    primitive carries an ordering effect, so executions serialize) for
    device-time measurement.
    """

    def __init__(self, n_cores=8):
        import jax

        from concourse import bass2jax, mybir

        bass2jax.install_neuronx_cc_hook()
        self.jax = jax
        self.bass2jax = bass2jax
        self.n_cores = n_cores
        self.nc = _build_nc()
        assert self.nc.dbg_addr is None
        self.partition_name = (
            self.nc.partition_id_tensor.name if self.nc.partition_id_tensor else None
        )

        in_names: list = []
        out_names: list = []
        out_avals: list = []
        zero_shapes: list = []
        for alloc in self.nc.m.functions[0].allocations:
            if not isinstance(alloc, mybir.MemoryLocationSet):
                continue
            name = alloc.memorylocations[0].name
            if alloc.kind == "ExternalInput":
                if name != self.partition_name:
                    in_names.append(name)
            elif alloc.kind == "ExternalOutput":
                shape = tuple(alloc.tensor_shape)
                dtype = mybir.dt.np(alloc.dtype)
                out_names.append(name)
                out_avals.append(jax.core.ShapedArray(shape, dtype))
                zero_shapes.append((shape, dtype))
        self.in_names = in_names
        self.out_names = out_names
        self.out_avals = out_avals
        self.zero_shapes = zero_shapes
        self._jits: dict = {}

    def _sharded(self, n_iters):
        if n_iters in self._jits:
            return self._jits[n_iters]
        jax = self.jax
        from jax.experimental.shard_map import shard_map
        from jax.sharding import Mesh, PartitionSpec

        n_params = len(self.in_names)
        n_outs = len(self.out_names)
        all_names = tuple(self.in_names) + tuple(self.out_names)
        if self.partition_name is not None:
            all_names = all_names + (self.partition_name,)
        out_avals = tuple(self.out_avals)
        nc = self.nc
        bind = self.bass2jax._bass_exec_p.bind
        partition_id_tensor = self.bass2jax.partition_id_tensor
        partition_name = self.partition_name

        def _body(*args):
            # n_iters > 1 reuses the same zero buffers for every bind so
            # each custom call's operand list matches the outer jit's
            # parameter order (neuronx_cc_hook requires it); the bass
            # effect keeps the executions ordered on each core.
            outs = None
            for _ in range(n_iters):
                operands = list(args)
                if partition_name is not None:
                    operands.append(partition_id_tensor())
                outs = bind(
                    *operands,
                    out_avals=out_avals,
                    in_names=all_names,
                    out_names=tuple(self.out_names),
                    lowering_input_output_aliases=(),
                    sim_require_finite=True,
                    sim_require_nnan=True,
                    nc=nc,
                )
            return tuple(outs)

        devices = jax.devices()[: self.n_cores]
        mesh = Mesh(np.asarray(devices), ("core",))
        n_args = n_params + n_outs
        donate = tuple(range(n_params, n_args)) if n_iters == 1 else ()
        sharded = jax.jit(
            shard_map(
                _body,
                mesh=mesh,
                in_specs=(PartitionSpec("core"),) * n_args,
                out_specs=(PartitionSpec("core"),) * n_outs,
                check_rep=False,
            ),
            donate_argnums=donate,
            keep_unused=True,
        )
        self._jits[n_iters] = sharded
        return sharded

    def run(self, in_maps, n_iters=1, as_numpy=True):
        n = self.n_cores
        concat_in = [
            np.concatenate([np.asarray(m[name]) for m in in_maps], axis=0)
            for name in self.in_names
        ]
        zeros = [
            np.zeros((n * sh[0], *sh[1:]), dt) for (sh, dt) in self.zero_shapes
        ]
        out_arrs = self._sharded(n_iters)(*concat_in, *zeros)
        if not as_numpy:
            return out_arrs
        return [
            {
                name: np.asarray(out_arrs[i]).reshape(n, *self.out_avals[i].shape)[c]
                for i, name in enumerate(self.out_names)
            }
            for c in range(n)
        ]


def _get_runner():
    if "runner" not in _NC_CACHE:
        _NC_CACHE["runner"] = _Runner()
    return _NC_CACHE["runner"]


def _shard_inputs(x, Wq, bq, Wk, bk, Wv, bv):
    in_maps = []
    for core in range(8):
        b = core % DP
        hg = core // DP
        sl = slice(D_LOC * hg, D_LOC * (hg + 1))
        in_maps.append(
            {
                "x": np.ascontiguousarray(x[b]),
                "wq": np.ascontiguousarray(Wq[sl]),
                "wk": np.ascontiguousarray(Wk[sl]),
                "wv": np.ascontiguousarray(Wv[sl]),
                "bq": np.ascontiguousarray(bq[sl]),
                "bk": np.ascontiguousarray(bk[sl]),
                "bv": np.ascontiguousarray(bv[sl]),
            }
        )
    return in_maps


def kernel(x, mask, Wq, bq, Wk, bk, Wv, bv):
    x = np.ascontiguousarray(np.asarray(x, dtype=np.float32))
    Wq = np.ascontiguousarray(np.asarray(Wq, dtype=np.float32))
    Wk = np.ascontiguousarray(np.asarray(Wk, dtype=np.float32))
    Wv = np.ascontiguousarray(np.asarray(Wv, dtype=np.float32))
    bq = np.ascontiguousarray(np.asarray(bq, dtype=np.float32))
    bk = np.ascontiguousarray(np.asarray(bk, dtype=np.float32))
    bv = np.ascontiguousarray(np.asarray(bv, dtype=np.float32))

    in_maps = _shard_inputs(x, Wq, bq, Wk, bk, Wv, bv)
    results = _get_runner().run(in_maps)

    out = np.empty((B, S, D), dtype=np.float32)
    for core in range(8):
        b = core % DP
        hg = core // DP
        out[b, :, D_LOC * hg : D_LOC * (hg + 1)] = results[core]["out"]
    return out


# revision 40
# speedup vs baseline: 63.1224x; 63.1224x over previous
"""Causal multi-head attention on 8 Trainium2 NeuronCores.

Problem (hardcoded): B=4, S=2048, D=1024, H=16, DH=64, fp32.
  q/k/v = x @ W.T + b ; heads split; scores = q k^T / sqrt(DH), causal
  mask, softmax, out = attn @ v, merge heads.

Sharding: data-parallel over batch (4) x tensor-parallel over head
groups (2).  Core c handles batch b = c % 4 and heads
[8*(c//4), 8*(c//4)+8).  Each core gets x[b] and the 512-row slice of
Wq/Wk/Wv (+bias) for its head group, returns out[b, :, 512*hg:+512].
No collectives needed; host scatters inputs / gathers outputs.

Per-core kernel design (Tile framework):
  - x^T and W^T built on-chip with PE transposes (fp32 has no DMA
    transpose path).
  - Projections computed in transposed form: Q^T,K^T = W^T.T @ x^T
    ([dout, s] layout) and V = x^T.T @ W^T ([s, dout] layout), with the
    bias folded in as an extra rank-1 matmul (ones vector x bias).
    Matmul inputs are bitcast to float32r: full-rate (1 cyc/row) on the
    PE at N>=512 vs 4 cyc/row for plain fp32.
  - Scores computed transposed, S^T[k, q] = K_h Q_h^T, one 128-row key
    tile x 1024-wide query chunk at a time, accumulated in PSUM.
  - Softmax without a max pass: scores ~ N(0,1) (max ~6 sigma over the
    whole tensor), so exp((q.k)*0.125) cannot overflow; softmax is
    shift-invariant so the result is exact.  The 1/sqrt(DH) scale rides
    the ACT activation's free scale.  Causal masking only affects the
    diagonal 128x128 block of each (key-tile, query-chunk) pair: a
    precomputed additive mask (-1e30 above... below diag) is added in
    PSUM.  exp() output is written as bf16 (attention weights in [0,
    e^6]) straight to SBUF.
  - attn @ V: attn^T tiles are the stationary operand (bf16 -> fast
    weight load), V tiles [128, 65] the moving operand, where column 64
    is ones so the PE accumulates the softmax denominator alongside.
    Output [q, 64+1] accumulates over key tiles in PSUM.
  - Finalize: reciprocal of column 64, per-partition scalar multiply,
    DMA out.
"""

import numpy as np

# Full problem shapes.
B, S, D, H, DH = 4, 2048, 1024, 16, 64
TP = 2
DP = 4
D_LOC = D // TP  # 512
H_LOC = H // TP  # 8

NEG = -1.0e30

# dtype for the fp32-ish matmuls: "float32r" (TF32-like, full PE rate at
# N>=256) or "float32" (exact, 4 cycles/row).
MM_DTYPE = "float32r"

# Knobs (test.py may override before first kernel() call).
RUN_OPTS: dict = {}
LAST_RESULT = None

_NC_CACHE: dict = {}



def _legalize_waits(nc, mybir):
    """Spill excess sync waits onto NoOps inserted before the instruction.

    Walrus enforces per-instruction sync-wait capacities (Matmult fuses
    LDWEIGHTS and has a single slot; most others have two).  Tile's wait
    assignment can exceed that when an instruction joins several
    semaphore domains.  Moving waits to a same-engine NoOp immediately
    before the instruction is semantics-preserving: the engine's
    sequencer executes them in order.
    """
    caps = {}
    ctr = [0]
    for fn in nc.m.functions:
        for blk in fn.blocks:
            insts = list(blk.instructions)
            out = []
            changed = False
            for inst in insts:
                si = inst.sync_info
                waits = list(si.on_wait) if si is not None and si.on_wait else []
                cap = caps.get(str(inst.opcode), 1)
                if len(waits) > cap:
                    excess = waits[: len(waits) - cap]
                    keep = waits[len(waits) - cap :]
                    for w in excess:
                        ev = mybir.InstEventSemaphore(
                            name=f"waitnop_{ctr[0]}",
                            opcode="EventSemaphore",
                            engine=inst.engine,
                            ins=[],
                            outs=[],
                            sync_info=mybir.SyncInfo(on_wait=[w], on_update=[]),
                        )
                        ctr[0] += 1
                        out.append(ev)
                    si.on_wait = keep
                    inst.sync_info = si
                    changed = True
                out.append(inst)
            if changed:
                blk.instructions = out
    return ctr[0]


def _build_nc(s=S, d_in=D, d_loc=D_LOC, h_loc=H_LOC, dh=DH, legalize=True, ablate="", cse_tag=0):
    """Build the per-core Bass program. All 8 cores run this SPMD."""
    from contextlib import ExitStack

    import concourse.bass as bass
    import concourse.mybir as mybir
    import concourse.tile as tile

    f32 = mybir.dt.float32
    f32r = getattr(mybir.dt, MM_DTYPE)
    bf16 = mybir.dt.bfloat16
    EXP = mybir.ActivationFunctionType.Exp

    assert s % 512 == 0 and d_in % 128 == 0 and d_loc % 128 == 0
    assert dh == 64 and d_loc == h_loc * dh
    KD = d_in // 128       # contraction k-tiles for projections
    NM = d_loc // 128      # dout m-tiles (4)
    NSB = s // 512         # s superblocks for projections
    NKT = s // 128         # key tiles (16)
    QC = min(1024, s)      # query chunk width
    NJC = s // QC          # query chunks (2)
    NJJ = QC // 128        # q-tiles per chunk (8)
    SCALE = 1.0 / float(np.sqrt(dh))

    nc = bass.Bass()

    # Transposed on the host: xt = x.T, w*t = W_slice.T.  Declared as
    # float32r (same 4-byte storage) so they can feed fp32r matmuls
    # straight from DMA.
    xt_d = nc.dram_tensor("xt", [d_in, s], f32r, kind="ExternalInput")
    wq_d = nc.dram_tensor("wqt", [d_in, d_loc], f32r, kind="ExternalInput")
    wk_d = nc.dram_tensor("wkt", [d_in, d_loc], f32r, kind="ExternalInput")
    wv_d = nc.dram_tensor("wvt", [d_in, d_loc], f32r, kind="ExternalInput")
    bq_d = nc.dram_tensor("bq", [d_loc], f32, kind="ExternalInput")
    bk_d = nc.dram_tensor("bk", [d_loc], f32, kind="ExternalInput")
    bv_d = nc.dram_tensor("bv", [d_loc], f32, kind="ExternalInput")
    out_d = nc.dram_tensor("out", [s, d_loc], f32, kind="ExternalOutput")

    import ml_dtypes

    # Multiplicative causal mask for the diagonal block of attn^T[k, q]:
    # valid (keep) where k <= q i.e. row <= col.
    mask_np = np.where(
        np.arange(128)[:, None] <= np.arange(128)[None, :], 1.0, 0.0
    ).astype(ml_dtypes.bfloat16)
    if cse_tag:
        # content marker so two otherwise-identical programs don't get
        # CSE'd when chained in one jit for timing
        nc.inline_tensor(np.full((1, 1), float(cse_tag), np.float32), name=f"csetag{cse_tag}")
    mask_dram = nc.inline_tensor(mask_np, name="cmask01")

    with tile.TileContext(nc) as tc, ExitStack() as ctx:
        persist = ctx.enter_context(tc.tile_pool(name="persist", bufs=1))
        proj_sb = ctx.enter_context(tc.tile_pool(name="proj_sb", bufs=1))
        proj_ps = ctx.enter_context(
            tc.tile_pool(name="proj_ps", bufs=1, space="PSUM")
        )

        # ---- constants ----
        cmask = persist.tile([128, 128], bf16)
        nc.sync.dma_start(out=cmask, in_=mask_dram[:])
        dve_scr = persist.tile([1, 8], f32)
        ones_st = persist.tile([1, 512], f32)
        nc.vector.memset(ones_st, 1.0)
        ones_r = persist.tile([1, 512], f32r)
        nc.vector.tensor_copy(out=ones_r, in_=ones_st)
        bias_st = persist.tile([1, 3, d_loc], f32)
        bias_sb = persist.tile([1, 3, d_loc], f32r)
        for i, b_d in enumerate((bq_d, bk_d, bv_d)):
            nc.sync.dma_start(out=bias_st[:, i, :], in_=b_d[:].unsqueeze(0))
            nc.vector.tensor_copy(out=bias_sb[:, i, :], in_=bias_st[:, i, :])

        # ---- persistent activations ----
        qt_sb = persist.tile([128, NM, s], f32r)      # Q^T  [dout, s]
        kt_sb = persist.tile([128, NM, s], f32r)      # K^T  [dout, s]
        v_sb = persist.tile([128, NKT, h_loc, dh + 1], bf16)  # V (+ones col)
        nc.vector.memset(v_sb, 1.0)

        # ---- W^T tiles: direct DMA of host-transposed weights ----
        wt_tiles = []
        for wi, w_d in enumerate((wq_d, wk_d, wv_d)):
            wt = proj_sb.tile(
                [128, KD, d_loc], f32r, name=f"wt{wi}", tag="wt", bufs=3
            )
            for kd in range(KD):
                nc.sync.dma_start(
                    out=wt[:, kd, :],
                    in_=w_d[128 * kd : 128 * (kd + 1), :],
                )
            wt_tiles.append(wt)
        wqt, wkt, wvt = wt_tiles

        # ---- projections, one 512-row superblock of s at a time ----
        def emit_proj(sb):
            xt = proj_sb.tile([128, KD, 512], f32r, name=f"xt{sb}", tag="xt", bufs=2)
            for kd in range(KD):
                nc.sync.dma_start(
                    out=xt[:, kd, :],
                    in_=xt_d[128 * kd : 128 * (kd + 1), 512 * sb : 512 * (sb + 1)],
                )

            # Q^T, K^T m-tiles: [dout 128, s 512] = sum_kd W^T.T @ x^T
            for wt, dest, bi in ((wqt, qt_sb, 0), (wkt, kt_sb, 1)):
                for m in range(NM):
                    ps = proj_ps.tile(
                        [128, 512], f32, name=f"psp{sb}_{bi}_{m}", tag="mm512", bufs=2
                    )
                    for kd in range(KD):
                        nc.tensor.matmul(
                            ps,
                            lhsT=wt[:, kd, 128 * m : 128 * (m + 1)],
                            rhs=xt[:, kd, :],
                            start=(kd == 0),
                            stop=False,
                        )
                    nc.tensor.matmul(
                        ps,
                        lhsT=bias_sb[:, bi, 128 * m : 128 * (m + 1)],
                        rhs=ones_r[:, :],
                        start=False,
                        stop=True,
                    )
                    nc.vector.tensor_copy(
                        out=dest[:, m, 512 * sb : 512 * (sb + 1)], in_=ps
                    )

            # V s-tiles: [s 128, dout 512] = sum_kd x^T.T @ W^T
            for t in range(4):
                kt_idx = 4 * sb + t
                ps = proj_ps.tile(
                    [128, d_loc], f32, name=f"psv{sb}_{t}", tag="mm512", bufs=2
                )
                for kd in range(KD):
                    nc.tensor.matmul(
                        ps,
                        lhsT=xt[:, kd, 128 * t : 128 * (t + 1)],
                        rhs=wvt[:, kd, :],
                        start=(kd == 0),
                        stop=False,
                    )
                nc.tensor.matmul(
                    ps,
                    lhsT=ones_r[:, 0:128],
                    rhs=bias_sb[:, 2, :],
                    start=False,
                    stop=True,
                )
                # strided copy into per-head [dh] slots (col dh stays ones)
                nc.vector.tensor_copy(
                    out=v_sb[:, kt_idx, :, 0:dh],
                    in_=ps.rearrange("p (h c) -> p h c", c=dh),
                )

        attn_sb = ctx.enter_context(tc.tile_pool(name="attn_sb", bufs=1))
        sc_ps_pool = ctx.enter_context(
            tc.tile_pool(name="sc_ps", bufs=1, space="PSUM")
        )
        oa_ps_pool = ctx.enter_context(
            tc.tile_pool(name="oa_ps", bufs=1, space="PSUM")
        )

        # ---- attention ----
        # Wait-budget bookkeeping (see comment at pe_touch): the scores
        # PSUM tile is read ONLY by the exp activation; the output
        # accumulator PSUM tile is read ONLY by one ACT copy; diagonal
        # masking happens post-exp on a separate bf16 tile so attn@V
        # matmuls join on a single semaphore (ACT for the plain tiles,
        # DVE for the masked diagonal tile).
        n_h = 0 if "noattn" in ablate else (1 if "attn1h" in ablate else h_loc)

        def emit_attn(jc):
            for h in range(n_h):
                pbase = 64 * (h % 2)
                mblk = h // 2
                i_max = NJJ * jc + (NJJ - 1)  # last key tile with any valid q
                oa_t = [
                    oa_ps_pool.tile(
                        [128, 260], f32, name=f"oa{jc}_{h}_{b}", tag="oa", bufs=2
                    )
                    for b in range(2)
                ]
                # per-PSUM-bank first/last matmul bookkeeping for start/stop.
                # Order i=0's matmuls non-diagonal-first so the first matmul
                # into each bank depends only on the ACT semaphore.
                def jj_order(i):
                    jj0 = max(0, i - NJJ * jc)
                    jd = i - NJJ * jc  # diagonal jj (may be out of range)
                    jjs = [j for j in range(jj0, NJJ) if j != jd]
                    if jj0 <= jd < NJJ:
                        pos = 1 if len(jjs) >= 1 else 0
                        jjs.insert(pos, jd)
                    return jjs

                mm_sched: dict = {}
                for i in range(i_max + 1):
                    for jj in jj_order(i):
                        mm_sched.setdefault(jj // 4, []).append((i, jj))
                first_mm = {b: v[0] for b, v in mm_sched.items()}
                last_mm = {b: v[-1] for b, v in mm_sched.items()}

                for i in range(i_max + 1):
                    jj0 = max(0, i - NJJ * jc)
                    jd = i - NJJ * jc
                    qv0 = 128 * jj0
                    sc = sc_ps_pool.tile(
                        [128, QC], f32, name=f"sc{jc}_{h}_{i}", tag="sc", bufs=2
                    )
                    kt_lhs = kt_sb[
                        pbase : pbase + dh,
                        mblk,
                        128 * i : 128 * (i + 1),
                    ]
                    for half in range(QC // 512):
                        if 512 * (half + 1) <= qv0:
                            continue  # fully masked half
                        nc.tensor.matmul(
                            sc[:, 512 * half : 512 * (half + 1)],
                            lhsT=kt_lhs,
                            rhs=qt_sb[
                                pbase : pbase + dh,
                                mblk,
                                QC * jc + 512 * half : QC * jc + 512 * (half + 1),
                            ],
                            start=True,
                            stop=True,
                        )
                    at = attn_sb.tile(
                        [128, QC], bf16, name=f"at{jc}_{h}_{i}", tag="at", bufs=4
                    )
                    nc.scalar.activation(
                        out=at[:, qv0:QC], in_=sc[:, qv0:QC],
                        func=(mybir.ActivationFunctionType.Copy
                              if "noexp" in ablate else EXP),
                        scale=SCALE,
                    )
                    # causal mask on the diagonal block (post-exp, bf16)
                    if jj0 <= jd < NJJ:
                        at_m = attn_sb.tile(
                            [128, 128], bf16, name=f"atm{jc}_{h}_{i}",
                            tag="atm", bufs=3,
                        )
                        nc.vector.tensor_mul(
                            out=at_m,
                            in0=at[:, 128 * jd : 128 * (jd + 1)],
                            in1=cmask,
                        )
                    vt = v_sb[:, i, h, :]  # [128, dh+1] bf16
                    for jj in jj_order(i):
                        bank = jj // 4
                        col = 65 * (jj % 4)
                        lhs = at_m if jj == jd else at[:, 128 * jj : 128 * (jj + 1)]
                        nc.tensor.matmul(
                            oa_t[bank][:, col : col + 65],
                            lhsT=lhs,
                            rhs=vt,
                            start=(first_mm[bank] == (i, jj)),
                            stop=(last_mm[bank] == (i, jj)),
                        )

                # finalize: one ACT copy drains the accumulator (sole oa
                # reader), then DVE divides by the denominator column.
                ot_raw = attn_sb.tile(
                    [128, 2, 4, dh + 1], f32, name=f"otr{jc}_{h}", tag="otr", bufs=2
                )
                for b in range(2):
                    nc.scalar.copy(
                        out=ot_raw[:, b],
                        in_=oa_t[b].rearrange("p (j c) -> p j c", c=65),
                    )
                ot = attn_sb.tile(
                    [128, NJJ, dh], f32, name=f"ot{jc}_{h}", tag="ot", bufs=4
                )
                for jj in range(NJJ):
                    rec = attn_sb.tile(
                        [128, 1], f32, name=f"rec{jc}_{h}_{jj}", tag="rec", bufs=4
                    )
                    nc.vector.reciprocal(
                        rec, ot_raw[:, jj // 4, jj % 4, dh : dh + 1]
                    )
                    nc.vector.tensor_scalar_mul(
                        out=ot[:, jj, :],
                        in0=ot_raw[:, jj // 4, jj % 4, 0:dh],
                        scalar1=rec,
                    )
                nc.sync.dma_start(
                    out=out_d[QC * jc : QC * (jc + 1), dh * h : dh * (h + 1)]
                    .rearrange("(jj p) c -> p jj c", p=128),
                    in_=ot,
                )

        # Interleaved emission: attention for query chunk jc only needs
        # projections of superblocks covering keys/queries < (jc+1)*QC,
        # so emit it before the later superblocks — the scheduler then
        # overlaps the ACT-bound attention with PE-bound projections.
        per_chunk = (QC // 512)
        for jc in range(NJC):
            for sb in range(per_chunk * jc, per_chunk * (jc + 1)):
                emit_proj(sb)
            emit_attn(jc)

    if legalize:
        _legalize_waits(nc, mybir)
    nc.finalize()
    return nc


class _Runner:
    """Caches the compiled SPMD executable across kernel() calls.

    Mirrors concourse.bass2jax.run_bass_via_pjrt's multi-core path, but
    keeps the jitted callable (and thus the NEFF executable) alive so
    repeated calls don't re-trace/re-compile.  Supports running the NEFF
    n_iters times back-to-back inside one jit call (the bass_exec
    primitive carries an ordering effect, so executions serialize) for
    device-time measurement.
    """

    def __init__(self, n_cores=8):
        import jax

        from concourse import bass2jax, mybir

        bass2jax.install_neuronx_cc_hook()
        self.jax = jax
        self.bass2jax = bass2jax
        self.n_cores = n_cores
        self.nc = _build_nc()
        assert self.nc.dbg_addr is None
        self.partition_name = (
            self.nc.partition_id_tensor.name if self.nc.partition_id_tensor else None
        )

        in_names: list = []
        out_names: list = []
        out_avals: list = []
        zero_shapes: list = []
        for alloc in self.nc.m.functions[0].allocations:
            if not isinstance(alloc, mybir.MemoryLocationSet):
                continue
            name = alloc.memorylocations[0].name
            if alloc.kind == "ExternalInput":
                if name != self.partition_name:
                    in_names.append(name)
            elif alloc.kind == "ExternalOutput":
                shape = tuple(alloc.tensor_shape)
                dtype = mybir.dt.np(alloc.dtype)
                out_names.append(name)
                out_avals.append(jax.core.ShapedArray(shape, dtype))
                zero_shapes.append((shape, dtype))
        self.in_names = in_names
        self.out_names = out_names
        self.out_avals = out_avals
        self.zero_shapes = zero_shapes
        self._jits: dict = {}

    def _sharded(self, n_iters, donate_zeros=True):
        key = (n_iters, donate_zeros)
        if key in self._jits:
            return self._jits[key]
        jax = self.jax
        from jax.experimental.shard_map import shard_map
        from jax.sharding import Mesh, PartitionSpec

        n_params = len(self.in_names)
        n_outs = len(self.out_names)
        all_names = tuple(self.in_names) + tuple(self.out_names)
        if self.partition_name is not None:
            all_names = all_names + (self.partition_name,)
        out_avals = tuple(self.out_avals)
        nc = self.nc
        bind = self.bass2jax._bass_exec_p.bind
        partition_id_tensor = self.bass2jax.partition_id_tensor
        partition_name = self.partition_name

        def _body(*args):
            # n_iters > 1 reuses the same zero buffers for every bind so
            # each custom call's operand list matches the outer jit's
            # parameter order (neuronx_cc_hook requires it); the bass
            # effect keeps the executions ordered on each core.
            outs = None
            for _ in range(n_iters):
                operands = list(args)
                if partition_name is not None:
                    operands.append(partition_id_tensor())
                outs = bind(
                    *operands,
                    out_avals=out_avals,
                    in_names=all_names,
                    out_names=tuple(self.out_names),
                    lowering_input_output_aliases=(),
                    sim_require_finite=True,
                    sim_require_nnan=True,
                    nc=nc,
                )
            return tuple(outs)

        devices = jax.devices()[: self.n_cores]
        mesh = Mesh(np.asarray(devices), ("core",))
        n_args = n_params + n_outs
        donate = tuple(range(n_params, n_args)) if donate_zeros else ()
        sharded = jax.jit(
            shard_map(
                _body,
                mesh=mesh,
                in_specs=(PartitionSpec("core"),) * n_args,
                out_specs=(PartitionSpec("core"),) * n_outs,
                check_rep=False,
            ),
            donate_argnums=donate,
            keep_unused=True,
        )
        self._jits[key] = sharded
        return sharded

    def device_args(self, in_maps):
        """device_put concat inputs + zeros once, correctly sharded."""
        import jax
        from jax.sharding import Mesh, NamedSharding, PartitionSpec

        n = self.n_cores
        mesh = Mesh(np.asarray(jax.devices()[:n]), ("core",))
        sh = NamedSharding(mesh, PartitionSpec("core"))
        concat_in = [
            np.concatenate([np.asarray(m[name]) for m in in_maps], axis=0)
            for name in self.in_names
        ]
        zeros = [
            np.zeros((n * s0[0], *s0[1:]), dt) for (s0, dt) in self.zero_shapes
        ]
        return [jax.device_put(a, sh) for a in concat_in + zeros]

    def bench(self, in_maps, reps=15, n_iters=1):
        """Min wall time of dispatch+n_iters execs, operands device-resident."""
        import time

        args = self.device_args(in_maps)
        fn = self._sharded(n_iters, donate_zeros=False)
        outs = fn(*args)
        for o in outs:
            o.block_until_ready()
        best = float("inf")
        for _ in range(reps):
            t0 = time.time()
            outs = fn(*args)
            for o in outs:
                o.block_until_ready()
            best = min(best, time.time() - t0)
        return best

    def run(self, in_maps, n_iters=1, as_numpy=True):
        n = self.n_cores
        concat_in = [
            np.concatenate([np.asarray(m[name]) for m in in_maps], axis=0)
            for name in self.in_names
        ]
        zeros = [
            np.zeros((n * sh[0], *sh[1:]), dt) for (sh, dt) in self.zero_shapes
        ]
        out_arrs = self._sharded(n_iters)(*concat_in, *zeros)
        if not as_numpy:
            return out_arrs
        return [
            {
                name: np.asarray(out_arrs[i]).reshape(n, *self.out_avals[i].shape)[c]
                for i, name in enumerate(self.out_names)
            }
            for c in range(n)
        ]


def _get_runner():
    if "runner" not in _NC_CACHE:
        _NC_CACHE["runner"] = _Runner()
    return _NC_CACHE["runner"]


def _shard_inputs(x, Wq, bq, Wk, bk, Wv, bv):
    # Host-side layout prep: the device kernel consumes x and W
    # transposed (contraction dim on partitions).
    xts = [np.ascontiguousarray(x[b].T) for b in range(DP)]
    wqt = np.ascontiguousarray(Wq.T)
    wkt = np.ascontiguousarray(Wk.T)
    wvt = np.ascontiguousarray(Wv.T)
    in_maps = []
    for core in range(8):
        b = core % DP
        hg = core // DP
        sl = slice(D_LOC * hg, D_LOC * (hg + 1))
        in_maps.append(
            {
                "xt": xts[b],
                "wqt": np.ascontiguousarray(wqt[:, sl]),
                "wkt": np.ascontiguousarray(wkt[:, sl]),
                "wvt": np.ascontiguousarray(wvt[:, sl]),
                "bq": np.ascontiguousarray(bq[sl]),
                "bk": np.ascontiguousarray(bk[sl]),
                "bv": np.ascontiguousarray(bv[sl]),
            }
        )
    return in_maps


def kernel(x, mask, Wq, bq, Wk, bk, Wv, bv):
    x = np.ascontiguousarray(np.asarray(x, dtype=np.float32))
    Wq = np.ascontiguousarray(np.asarray(Wq, dtype=np.float32))
    Wk = np.ascontiguousarray(np.asarray(Wk, dtype=np.float32))
    Wv = np.ascontiguousarray(np.asarray(Wv, dtype=np.float32))
    bq = np.ascontiguousarray(np.asarray(bq, dtype=np.float32))
    bk = np.ascontiguousarray(np.asarray(bk, dtype=np.float32))
    bv = np.ascontiguousarray(np.asarray(bv, dtype=np.float32))

    in_maps = _shard_inputs(x, Wq, bq, Wk, bk, Wv, bv)
    results = _get_runner().run(in_maps)

    out = np.empty((B, S, D), dtype=np.float32)
    for core in range(8):
        b = core % DP
        hg = core // DP
        out[b, :, D_LOC * hg : D_LOC * (hg + 1)] = results[core]["out"]
    return out
